# revision 57
# baseline (speedup 1.0000x reference)
"""Trainium2 Bass kernel for DecoderMultiHeadAttention, sharded over 8 cores.

Sharding: core c handles batch b=c//4 and head-group g=c%4 (4 of 16 heads).
Each core computes q/k/v projections for its heads, masked softmax attention
(transpose-free: logits computed as logitsT[j,i] so the softmax reduction is
along the matmul contraction), and a partial W_o projection summed over its
heads. Host sums the 4 partial outputs per batch.

All matmul operands are bf16 (PSUM accumulation stays fp32): 16-bit weights
enable FWL fast weight load and hidden LDWEIGHTS, and bf16 enables the DVE
2x packed mode for the mask multiplies. DMA count is minimized (resident xe,
quad-tile xd/mask) because each dma_start costs ~680ns of serialized issue
time on the sync sequencer. The attention jt-loop is software-pipelined
(next tile's QK matmuls are emitted before this tile's PV matmuls) so the
PE isn't queue-blocked behind the exp/mask chain, and the W_o projection is
interleaved into the second attention pass to hide under the scalar-bound
softmax stretch.

Self-contained: hardcodes all shapes from the problem spec.
"""

import os
import sys
import types

import numpy as np

# ---------------------------------------------------------------------------
# Environment shims (axon NTFF hook registry + no-op artifact upload)
# ---------------------------------------------------------------------------


def _install_shims():
    if "antenv.axon_hooks" not in sys.modules:
        mod = types.ModuleType("antenv.axon_hooks")
        _hook = [None]
        mod.set_axon_ntff_profile_hook = lambda h: _hook.__setitem__(0, h)
        mod.get_axon_ntff_profile_hook = lambda: _hook[0]
        sys.modules["antenv.axon_hooks"] = mod
        try:
            import antenv

            antenv.axon_hooks = mod
        except Exception:
            pass
        try:
            from trn_agent_boot.trn_boot import _ntff_profile_via_ctypes

            mod.set_axon_ntff_profile_hook(
                _ntff_profile_via_ctypes("/opt/axon/libaxon_pjrt.so")
            )
        except Exception:
            pass
    try:
        import concourse.bass_utils as bass_utils

        bass_utils.upload_artifacts = lambda tmpdir: f"file://{tmpdir}"
    except Exception:
        pass


_install_shims()

from contextlib import ExitStack

import concourse.bass as bass
import concourse.tile as tile
from concourse import bacc, mybir
import concourse.bass_utils as bass_utils

f32 = mybir.dt.float32
bf16 = mybir.dt.bfloat16

# Problem constants
N_BATCH = 2
S = 2048
E = 1024
H = 16
HD = 64
NC = 8
G = 4  # head groups (one per core within a batch)
NEG_SCALE = 0.125  # 1/sqrt(hd)
EXP_SHIFT = -5.0  # constant pre-exp shift (cancels in softmax normalization)

JT = S // 128  # 16 j-tiles (key index)
IB = S // 512  # 4 i-blocks (query index)
ES = E // 128  # 8 E slices
ET = E // 128  # 8 output-channel tiles

WO_JTS = (4, 5, 7, 8, 10, 11, 13, 14)  # jt slots for interleaved W_o units


def build_nc():
    nc = bacc.Bacc(
        "TRN2",
        target_bir_lowering=False,
        debug=False,
        enable_asserts=False,
        num_devices=NC,
    )
    xeT_d = nc.dram_tensor("xeT", [E, S], bf16, kind="ExternalInput").ap()
    xdT_d = nc.dram_tensor("xdT", [E, S], bf16, kind="ExternalInput").ap()
    wqkT_d = nc.dram_tensor("wqkT", [E, 512], bf16, kind="ExternalInput").ap()
    wvT_d = nc.dram_tensor("wvT", [E, 256], bf16, kind="ExternalInput").ap()
    woT_d = nc.dram_tensor("woT", [256, E], bf16, kind="ExternalInput").ap()
    maskT_d = nc.dram_tensor("maskT", [S, S], bf16, kind="ExternalInput").ap()
    outT_d = nc.dram_tensor("outT", [E, S], bf16, kind="ExternalOutput").ap()
    # DRAM bounce buffers for the partition-broadcast of 1/rowsum
    rs_scr = [
        nc.dram_tensor(f"rs_scr{i}", [1, 1024], f32, kind="Internal").ap()
        for i in range(8)
    ]

    with tile.TileContext(nc) as tc, ExitStack() as ctx:
        const = ctx.enter_context(tc.tile_pool(name="const", bufs=1))
        xd_pool = ctx.enter_context(tc.tile_pool(name="xd", bufs=16))
        ex_pool = ctx.enter_context(tc.tile_pool(name="ex", bufs=6))
        out_pool = ctx.enter_context(tc.tile_pool(name="outp", bufs=3))
        div_pool = ctx.enter_context(tc.tile_pool(name="divp", bufs=2))
        qk_ps = ctx.enter_context(tc.tile_pool(name="qkps", bufs=2, space="PSUM"))
        pv_ps = ctx.enter_context(tc.tile_pool(name="pvps", bufs=2, space="PSUM"))
        po_ps = ctx.enter_context(tc.tile_pool(name="pops", bufs=2, space="PSUM"))

        Copy = mybir.ActivationFunctionType.Copy

        # ---- static SBUF tensors -----------------------------------------
        exp_bias = const.tile([128, 1], f32, name="exp_bias")
        nc.vector.memset(exp_bias[:], EXP_SHIFT)
        # qk weights first (phase B critical), then encoder activations in
        # half-tiles so the first i-halves land quickly, then wv / wo.
        wqk_sb = []
        for es in range(ES):
            wq = const.tile([128, 512], bf16, name=f"wqk{es}")
            nc.sync.dma_start(wq[:], wqkT_d[es * 128 : (es + 1) * 128, :])
            wqk_sb.append(wq)
        xe_sb = []
        for es in range(ES):
            xt = const.tile([128, S], bf16, name=f"xe{es}")
            nc.sync.dma_start(
                xt[:, 0:1024], xeT_d[es * 128 : (es + 1) * 128, 0:1024]
            )
            xe_sb.append(xt)
        for es in range(ES):
            nc.sync.dma_start(
                xe_sb[es][:, 1024:2048], xeT_d[es * 128 : (es + 1) * 128, 1024:2048]
            )
        wv_sb = []
        for es in range(ES):
            wv = const.tile([128, 256], bf16, name=f"wv{es}")
            nc.sync.dma_start(wv[:], wvT_d[es * 128 : (es + 1) * 128, :])
            wv_sb.append(wv)
        wo_sb = []
        for hp in range(2):
            wo = const.tile([128, E], bf16, name=f"wo{hp}")
            nc.sync.dma_start(wo[:], woT_d[hp * 128 : (hp + 1) * 128, :])
            wo_sb.append(wo)
        # mask tiles allocated here, DMA'd during/after phase C so the
        # projection phases aren't starved behind 8MB of mask traffic
        mask_sb = []
        for jt in range(JT):
            mt = const.tile([128, S], bf16, name=f"mask{jt}")
            mask_sb.append(mt)

        k_sb = []
        q_sb = []
        vals_sb = []
        for hp in range(2):
            kt = const.tile([128, S], bf16, name=f"ksb{hp}")
            qt = const.tile([128, S], bf16, name=f"qsb{hp}")
            vt = const.tile([128, S], bf16, name=f"valssb{hp}")
            k_sb.append(kt)
            q_sb.append(qt)
            vals_sb.append(vt)
        # v tiles padded to 128 cols per head (64 v chans, ones col at 64,
        # zeros elsewhere) so the PV lhsT has NumWeights=128 and gets FWL
        v_sb = []
        for jt in range(JT):
            vt = const.tile([128, 4 * 128], bf16, name=f"vsb{jt}")
            v_sb.append(vt)

        # ---- phase B: q/k projection -------------------------------------
        # qkT[c, s] = sum_e wqkT[e, c] * xeT[e, s]; chan tiles:
        #   ct0=[k_h0;k_h1] ct1=[q_h0;q_h1] ct2=[k_h2;k_h3] ct3=[q_h2;q_h3]
        for sb_i in range(4):
            pa = qk_ps.tile([128, 1024], f32, name="projps_a", tag="qkps")
            ssl = slice(sb_i * 512, sb_i * 512 + 512)
            for es in range(ES):
                for ct in range(2):
                    nc.tensor.matmul(
                        pa[:, ct * 512 : ct * 512 + 512],
                        lhsT=wqk_sb[es][:, ct * 128 : (ct + 1) * 128],
                        rhs=xe_sb[es][:, ssl],
                        start=(es == 0),
                        stop=(es == ES - 1),
                    )
            # drains split scalar/vector (scalar is idle outside attention)
            nc.scalar.activation(k_sb[0][:, ssl], pa[:, 0:512], Copy)
            nc.vector.tensor_copy(q_sb[0][:, ssl], pa[:, 512:1024])

        def b1_unit(u):
            # one hp=1 projection unit (ct 2=k, 3=q for s-block u//2),
            # interleaved into the hp=0 attention pass's PE slack
            ct, sb_i = 2 + u % 2, u // 2
            ssl = slice(sb_i * 512, sb_i * 512 + 512)
            pk = po_ps.tile([128, 512], f32, name="wops", tag="pops")
            for es in range(ES):
                nc.tensor.matmul(
                    pk[:],
                    lhsT=wqk_sb[es][:, ct * 128 : (ct + 1) * 128],
                    rhs=xe_sb[es][:, ssl],
                    start=(es == 0),
                    stop=(es == ES - 1),
                )
            dst = k_sb[1] if ct == 2 else q_sb[1]
            nc.vector.tensor_copy(dst[:, ssl], pk[:])

        # ---- phase C: v projection ---------------------------------------
        # v[s, c] = sum_e xdT[e, s] * wvT[e, c]; store interleaved (h, 128)
        # with a ones column at [h, 64] for the softmax denominator.
        # xd streamed in quad-tiles [128, 512] covering 4 s-tiles each.
        for stq in range(4):
            xd_q = []
            for es in range(ES):
                xt = xd_pool.tile([128, 512], bf16, name="xdt")
                nc.sync.dma_start(
                    xt[:],
                    xdT_d[es * 128 : (es + 1) * 128, stq * 512 : stq * 512 + 512],
                )
                xd_q.append(xt)
            # trickle in the first mask i-halves so attention isn't gated on
            # mask arrival when phase C finishes
            for jt in range(stq * 4, stq * 4 + 4):
                nc.sync.dma_start(
                    mask_sb[jt][:, 0:1024], maskT_d[jt * 128 : (jt + 1) * 128, 0:1024]
                )
            for k in range(4):
                st = stq * 4 + k
                pv = pv_ps.tile([128, 256], f32, name="vprojps", tag="pvps")
                for es in range(ES):
                    nc.tensor.matmul(
                        pv[:],
                        lhsT=xd_q[es][:, k * 128 : (k + 1) * 128],
                        rhs=wv_sb[es][:],
                        start=(es == 0),
                        stop=(es == ES - 1),
                    )
                view = v_sb[st].rearrange("p (h c) -> p h c", c=128)
                if st % 2 == 0:
                    nc.vector.tensor_copy(
                        view[:, :, 0:64], pv[:].rearrange("p (h c) -> p h c", c=64)
                    )
                else:
                    nc.scalar.activation(
                        view[:, :, 0:64],
                        pv[:].rearrange("p (h c) -> p h c", c=64),
                        Copy,
                    )
                nc.gpsimd.memset(view[:, :, 64:65], 1.0)
                nc.gpsimd.memset(view[:, :, 65:128], 0.0)

        # second mask i-halves (needed from the ib=2 attention pass onward)
        for jt in range(JT):
            nc.sync.dma_start(
                mask_sb[jt][:, 1024:2048], maskT_d[jt * 128 : (jt + 1) * 128, 1024:2048]
            )

        # ---- phase D: attention (+ phase E interleaved in hp=1 pass) -----
        def wo_unit(et, ib2, split_dma=False):
            po = po_ps.tile([128, 512], f32, name="wops", tag="pops")
            for hp2 in range(2):
                nc.tensor.matmul(
                    po[:],
                    lhsT=wo_sb[hp2][:, et * 128 : (et + 1) * 128],
                    rhs=vals_sb[hp2][:, ib2 * 512 : ib2 * 512 + 512],
                    start=(hp2 == 0),
                    stop=(hp2 == 1),
                )
            ot = out_pool.tile([128, 512], bf16, name="ot")
            nc.vector.tensor_copy(ot[:], po[:])
            nc.sync.dma_start(
                outT_d[et * 128 : (et + 1) * 128, ib2 * 512 : ib2 * 512 + 512],
                ot[:],
            )

        def qk_mms(hp, ib, jt):
            isl = slice(ib * 512, ib * 512 + 512)
            jsl = slice(jt * 128, jt * 128 + 128)
            qk_t = qk_ps.tile([128, 1024], f32, name="qkt", tag="qkps")
            nc.tensor.matmul(
                qk_t[:, 0:512],
                lhsT=k_sb[hp][0:64, jsl],
                rhs=q_sb[hp][0:64, isl],
                start=True,
                stop=True,
                tile_position=(0, 0),
            )
            nc.tensor.matmul(
                qk_t[:, 512:1024],
                lhsT=k_sb[hp][64:128, jsl],
                rhs=q_sb[hp][64:128, isl],
                start=True,
                stop=True,
                tile_position=(64, 0),
            )
            return qk_t

        for hp in range(2):
            for ib in range(IB):
                isl = slice(ib * 512, ib * 512 + 512)
                pvA = pv_ps.tile([128, 512], f32, name="pvA", tag="pvps")
                pvB = pv_ps.tile([128, 512], f32, name="pvB", tag="pvps")
                for jt in range(JT):
                    qk_t = qk_mms(hp, ib, jt)
                    ex_t = ex_pool.tile([128, 1024], bf16, name="ext")
                    nc.scalar.activation(
                        ex_t[:],
                        qk_t[:],
                        mybir.ActivationFunctionType.Exp,
                        bias=exp_bias[:],
                        scale=NEG_SCALE,
                    )
                    exv = ex_t.rearrange("p (h c) -> p h c", c=512)
                    nc.vector.tensor_mul(
                        exv[:, :, :],
                        exv[:, :, :],
                        mask_sb[jt][:, isl].rearrange("p (h c) -> p h c", c=512)
                        .broadcast_to([128, 2, 512]),
                    )
                    vview = v_sb[jt].rearrange("p (h c) -> p h c", c=128)
                    nc.tensor.matmul(
                        pvA[:],
                        lhsT=vview[:, 2 * hp, :],
                        rhs=ex_t[:, 0:512],
                        start=(jt == 0),
                        stop=(jt == JT - 1),
                    )
                    nc.tensor.matmul(
                        pvB[:],
                        lhsT=vview[:, 2 * hp + 1, :],
                        rhs=ex_t[:, 512:1024],
                        start=(jt == 0),
                        stop=(jt == JT - 1),
                    )
                    # interleave filler into the PE slack under the
                    # scalar-bound softmax: hp=1 projection units during
                    # hp=0, previous i-block's W_o units during hp=1
                    if hp == 0 and jt in (5, 10):
                        b1_unit(ib * 2 + (0 if jt == 5 else 1))
                    if hp == 1 and ib > 0 and jt in WO_JTS:
                        wo_unit(WO_JTS.index(jt), ib - 1)
                # normalize: vals = pv[0:65] / pv[64]. First drain pv[0:65]
                # to SBUF (scalar head A, vector head B) so the psum banks
                # free up immediately and the next i-block's PV accumulation
                # isn't stalled on the normalization chain.
                pvsA = div_pool.tile([128, 512], f32, name="pvsA")
                pvsB = div_pool.tile([128, 512], f32, name="pvsB")
                nc.scalar.activation(pvsA[0:65, :], pvA[0:65, :], Copy)
                nc.vector.tensor_copy(pvsB[0:65, :], pvB[0:65, :])
                # rowsums live at SBUF partition 64; bounce through DRAM to
                # broadcast across partitions (SBUF sources can't have a
                # zero partition stride), take reciprocal, then multiply
                # (the head-B result is moved to partitions 64..127 by DMA,
                # which shifts freely).
                scr = rs_scr[hp * IB + ib]
                nc.sync.dma_start(scr[0:1, 0:512], pvsA[64:65, :])
                nc.sync.dma_start(scr[0:1, 512:1024], pvsB[64:65, :])
                for h2, pvs in ((0, pvsA), (1, pvsB)):
                    rb = div_pool.tile([64, 512], f32, name="rb")
                    nc.sync.dma_start(
                        rb[:],
                        scr[0:1, h2 * 512 : h2 * 512 + 512].broadcast_to([64, 512]),
                    )
                    rc = div_pool.tile([64, 512], f32, name="rc")
                    nc.vector.reciprocal_approx_fast(rc[:], rb[:])
                    if h2 == 0:
                        nc.vector.tensor_mul(
                            vals_sb[hp][0:64, isl], pvs[0:64, :], rc[:]
                        )
                    else:
                        vtmp = div_pool.tile([64, 512], bf16, name="vtmp")
                        nc.vector.tensor_mul(vtmp[:], pvs[0:64, :], rc[:])
                        nc.sync.dma_start(
                            vals_sb[hp][64:128, isl], vtmp[:]
                        )

        # ---- phase E remainder: W_o for the last i-block -----------------
        # double-buffered via the (now free) qk psum pool, drains split
        # scalar/vector so the tail isn't serialized on one engine
        lsl = slice((IB - 1) * 512, IB * 512)

        def rem_mm1(et):
            # hp0 half of a remainder unit: depends only on long-final hp0
            # values, so it runs immediately after the attention loop and
            # keeps the PE inside the HAM busy window while the last
            # normalization chain completes
            po2 = qk_ps.tile([128, 1024], f32, name="wops2", tag="qkps")
            nc.tensor.matmul(
                po2[:, 0:512],
                lhsT=wo_sb[0][:, et * 128 : (et + 1) * 128],
                rhs=vals_sb[0][:, lsl],
                start=True,
                stop=False,
            )
            return po2

        po2s = {0: rem_mm1(0), 1: rem_mm1(1)}
        for et in range(ET):
            po2 = po2s.pop(et)
            nc.tensor.matmul(
                po2[:, 0:512],
                lhsT=wo_sb[1][:, et * 128 : (et + 1) * 128],
                rhs=vals_sb[1][:, lsl],
                start=False,
                stop=True,
            )
            if et + 2 < ET:
                po2s[et + 2] = rem_mm1(et + 2)
            ot = out_pool.tile([128, 512], bf16, name="ot")
            if et % 2 == 0:
                nc.vector.tensor_copy(ot[:], po2[:, 0:512])
            else:
                nc.scalar.activation(ot[:], po2[:, 0:512], Copy)
            nc.sync.dma_start(
                outT_d[et * 128 : (et + 1) * 128, lsl],
                ot[:],
            )

    nc.compile()
    return nc


_NC_CACHE = None


def _get_nc():
    global _NC_CACHE
    if _NC_CACHE is None:
        _NC_CACHE = build_nc()
    return _NC_CACHE


def _bf16(a):
    try:
        import ml_dtypes

        return np.asarray(a, dtype=ml_dtypes.bfloat16)
    except ImportError:
        import jax.numpy as jnp

        return np.asarray(jnp.asarray(a, dtype=jnp.bfloat16))


def shard_inputs(x_encoder, x_decoder, mask, W_qk, W_v, W_o):
    """Build the 8 per-core input maps."""
    x_encoder = np.asarray(x_encoder)
    x_decoder = np.asarray(x_decoder)
    mask = np.asarray(mask)
    W_qk = np.asarray(W_qk)
    W_v = np.asarray(W_v)
    W_o = np.asarray(W_o)

    xeT = [_bf16(np.ascontiguousarray(x_encoder[b].T)) for b in range(N_BATCH)]
    xdT = [_bf16(np.ascontiguousarray(x_decoder[b].T)) for b in range(N_BATCH)]
    maskT = _bf16(np.ascontiguousarray(mask.T))

    wqkT_g = []
    wvT_g = []
    woT_g = []
    for g in range(G):
        rows = []
        for hp in range(2):
            h0 = 4 * g + 2 * hp
            h1 = h0 + 1
            # k chans for the pair, then q chans (matches ct order)
            rows.append(W_qk[128 * h0 + 64 : 128 * h0 + 128])
            rows.append(W_qk[128 * h1 + 64 : 128 * h1 + 128])
            rows.append(W_qk[128 * h0 : 128 * h0 + 64])
            rows.append(W_qk[128 * h1 : 128 * h1 + 64])
        # reorder so layout is [k0,k1 | q0,q1 | k2,k3 | q2,q3] along cols
        sel = np.concatenate(
            [rows[0], rows[1], rows[2], rows[3], rows[4], rows[5], rows[6], rows[7]],
            axis=0,
        )
        wqkT_g.append(_bf16(np.ascontiguousarray(sel.T)))
        wvT_g.append(_bf16(np.ascontiguousarray(W_v[256 * g : 256 * g + 256, :].T)))
        woT_g.append(_bf16(np.ascontiguousarray(W_o[:, 256 * g : 256 * g + 256].T)))

    in_maps = []
    for c in range(NC):
        b, g = c // G, c % G
        in_maps.append(
            {
                "xeT": xeT[b],
                "xdT": xdT[b],
                "wqkT": wqkT_g[g],
                "wvT": wvT_g[g],
                "woT": woT_g[g],
                "maskT": maskT,
            }
        )
    return in_maps


def gather_outputs(results):
    """Sum per-group partials and transpose back to (N, S, E)."""
    out = np.empty((N_BATCH, S, E), dtype=np.float32)
    for b in range(N_BATCH):
        acc = results[b * G]["outT"].astype(np.float32)
        for g in range(1, G):
            acc += results[b * G + g]["outT"].astype(np.float32)
        out[b] = acc.T
    return out


def kernel(x_encoder, x_decoder, mask, W_qk, W_v, W_o):
    nc = _get_nc()
    in_maps = shard_inputs(x_encoder, x_decoder, mask, W_qk, W_v, W_o)
    res = bass_utils.run_bass_kernel_spmd(
        nc, in_maps, core_ids=list(range(NC)), trace=False
    )
    kernel.last_results = res
    return gather_outputs(res.results)


# revision 58
# speedup vs baseline: 1.0206x; 1.0206x over previous
"""Trainium2 Bass kernel for DecoderMultiHeadAttention, sharded over 8 cores.

Sharding: core c handles batch b=c//4 and head-group g=c%4 (4 of 16 heads).
Each core computes q/k/v projections for its heads, masked softmax attention
(transpose-free: logits computed as logitsT[j,i] so the softmax reduction is
along the matmul contraction), and a partial W_o projection summed over its
heads. Host sums the 4 partial outputs per batch.

All matmul operands are bf16 (PSUM accumulation stays fp32): 16-bit weights
enable FWL fast weight load and hidden LDWEIGHTS, and bf16 enables the DVE
2x packed mode for the mask multiplies. DMA count is minimized (resident xe,
quad-tile xd/mask) because each dma_start costs ~680ns of serialized issue
time on the sync sequencer. The attention jt-loop is software-pipelined
(next tile's QK matmuls are emitted before this tile's PV matmuls) so the
PE isn't queue-blocked behind the exp/mask chain, and the W_o projection is
interleaved into the second attention pass to hide under the scalar-bound
softmax stretch.

Self-contained: hardcodes all shapes from the problem spec.
"""

import os
import sys
import types

import numpy as np

# ---------------------------------------------------------------------------
# Environment shims (axon NTFF hook registry + no-op artifact upload)
# ---------------------------------------------------------------------------


def _install_shims():
    if "antenv.axon_hooks" not in sys.modules:
        mod = types.ModuleType("antenv.axon_hooks")
        _hook = [None]
        mod.set_axon_ntff_profile_hook = lambda h: _hook.__setitem__(0, h)
        mod.get_axon_ntff_profile_hook = lambda: _hook[0]
        sys.modules["antenv.axon_hooks"] = mod
        try:
            import antenv

            antenv.axon_hooks = mod
        except Exception:
            pass
        try:
            from trn_agent_boot.trn_boot import _ntff_profile_via_ctypes

            mod.set_axon_ntff_profile_hook(
                _ntff_profile_via_ctypes("/opt/axon/libaxon_pjrt.so")
            )
        except Exception:
            pass
    try:
        import concourse.bass_utils as bass_utils

        bass_utils.upload_artifacts = lambda tmpdir: f"file://{tmpdir}"
    except Exception:
        pass


_install_shims()

from contextlib import ExitStack

import concourse.bass as bass
import concourse.tile as tile
from concourse import bacc, mybir
import concourse.bass_utils as bass_utils

f32 = mybir.dt.float32
bf16 = mybir.dt.bfloat16

# Problem constants
N_BATCH = 2
S = 2048
E = 1024
H = 16
HD = 64
NC = 8
G = 4  # head groups (one per core within a batch)
NEG_SCALE = 0.125  # 1/sqrt(hd)
EXP_SHIFT = -5.0  # constant pre-exp shift (cancels in softmax normalization)

JT = S // 128  # 16 j-tiles (key index)
IB = S // 512  # 4 i-blocks (query index)
ES = E // 128  # 8 E slices
ET = E // 128  # 8 output-channel tiles

WO_JTS = (4, 5, 7, 8, 10, 11, 13, 14)  # jt slots for interleaved W_o units


def build_nc():
    nc = bacc.Bacc(
        "TRN2",
        target_bir_lowering=False,
        debug=False,
        enable_asserts=False,
        num_devices=NC,
    )
    xeT_d = nc.dram_tensor("xeT", [E, S], bf16, kind="ExternalInput").ap()
    xdT_d = nc.dram_tensor("xdT", [E, S], bf16, kind="ExternalInput").ap()
    wqkT_d = nc.dram_tensor("wqkT", [E, 512], bf16, kind="ExternalInput").ap()
    wvT_d = nc.dram_tensor("wvT", [E, 256], bf16, kind="ExternalInput").ap()
    woT_d = nc.dram_tensor("woT", [256, E], bf16, kind="ExternalInput").ap()
    maskT_d = nc.dram_tensor("maskT", [S, S], bf16, kind="ExternalInput").ap()
    outT_d = nc.dram_tensor("outT", [E, S], bf16, kind="ExternalOutput").ap()
    # DRAM bounce buffers for the partition-broadcast of 1/rowsum
    rs_scr = [
        nc.dram_tensor(f"rs_scr{i}", [1, 1024], f32, kind="Internal").ap()
        for i in range(8)
    ]

    with tile.TileContext(nc) as tc, ExitStack() as ctx:
        const = ctx.enter_context(tc.tile_pool(name="const", bufs=1))
        xd_pool = ctx.enter_context(tc.tile_pool(name="xd", bufs=16))
        ex_pool = ctx.enter_context(tc.tile_pool(name="ex", bufs=6))
        out_pool = ctx.enter_context(tc.tile_pool(name="outp", bufs=3))
        div_pool = ctx.enter_context(tc.tile_pool(name="divp", bufs=2))
        qk_ps = ctx.enter_context(tc.tile_pool(name="qkps", bufs=2, space="PSUM"))
        pv_ps = ctx.enter_context(tc.tile_pool(name="pvps", bufs=2, space="PSUM"))
        po_ps = ctx.enter_context(tc.tile_pool(name="pops", bufs=2, space="PSUM"))

        Copy = mybir.ActivationFunctionType.Copy

        # ---- static SBUF tensors -----------------------------------------
        exp_bias = const.tile([128, 1], f32, name="exp_bias")
        nc.vector.memset(exp_bias[:], EXP_SHIFT)
        # qk weights first (phase B critical), then encoder activations in
        # half-tiles so the first i-halves land quickly, then wv / wo.
        wqk_sb = []
        for es in range(ES):
            wq = const.tile([128, 512], bf16, name=f"wqk{es}")
            nc.sync.dma_start(wq[:], wqkT_d[es * 128 : (es + 1) * 128, :])
            wqk_sb.append(wq)
        xe_sb = []
        for es in range(ES):
            xt = const.tile([128, S], bf16, name=f"xe{es}")
            nc.sync.dma_start(
                xt[:, 0:1024], xeT_d[es * 128 : (es + 1) * 128, 0:1024]
            )
            xe_sb.append(xt)
        for es in range(ES):
            nc.sync.dma_start(
                xe_sb[es][:, 1024:2048], xeT_d[es * 128 : (es + 1) * 128, 1024:2048]
            )
        wv_sb = []
        for es in range(ES):
            wv = const.tile([128, 256], bf16, name=f"wv{es}")
            nc.sync.dma_start(wv[:], wvT_d[es * 128 : (es + 1) * 128, :])
            wv_sb.append(wv)
        wo_sb = []
        for hp in range(2):
            wo = const.tile([128, E], bf16, name=f"wo{hp}")
            nc.sync.dma_start(wo[:], woT_d[hp * 128 : (hp + 1) * 128, :])
            wo_sb.append(wo)
        # mask tiles allocated here, DMA'd during/after phase C so the
        # projection phases aren't starved behind 8MB of mask traffic
        mask_sb = []
        for jt in range(JT):
            mt = const.tile([128, S], bf16, name=f"mask{jt}")
            mask_sb.append(mt)

        k_sb = []
        q_sb = []
        vals_sb = []
        for hp in range(2):
            kt = const.tile([128, S], bf16, name=f"ksb{hp}")
            qt = const.tile([128, S], bf16, name=f"qsb{hp}")
            vt = const.tile([128, S], bf16, name=f"valssb{hp}")
            k_sb.append(kt)
            q_sb.append(qt)
            vals_sb.append(vt)
        # v tiles padded to 128 cols per head (64 v chans, ones col at 64,
        # zeros elsewhere) so the PV lhsT has NumWeights=128 and gets FWL
        v_sb = []
        for jt in range(JT):
            vt = const.tile([128, 4 * 128], bf16, name=f"vsb{jt}")
            v_sb.append(vt)

        # ---- phase B: q/k projection -------------------------------------
        # qkT[c, s] = sum_e wqkT[e, c] * xeT[e, s]; chan tiles:
        #   ct0=[k_h0;k_h1] ct1=[q_h0;q_h1] ct2=[k_h2;k_h3] ct3=[q_h2;q_h3]
        for sb_i in range(4):
            pa = qk_ps.tile([128, 1024], f32, name="projps_a", tag="qkps")
            ssl = slice(sb_i * 512, sb_i * 512 + 512)
            for es in range(ES):
                for ct in range(2):
                    nc.tensor.matmul(
                        pa[:, ct * 512 : ct * 512 + 512],
                        lhsT=wqk_sb[es][:, ct * 128 : (ct + 1) * 128],
                        rhs=xe_sb[es][:, ssl],
                        start=(es == 0),
                        stop=(es == ES - 1),
                    )
            # drains split scalar/vector (scalar is idle outside attention)
            nc.scalar.activation(k_sb[0][:, ssl], pa[:, 0:512], Copy)
            nc.vector.tensor_copy(q_sb[0][:, ssl], pa[:, 512:1024])

        def b1_unit(u):
            # one hp=1 projection unit (ct 2=k, 3=q for s-block u//2),
            # interleaved into the hp=0 attention pass's PE slack
            ct, sb_i = 2 + u % 2, u // 2
            ssl = slice(sb_i * 512, sb_i * 512 + 512)
            pk = po_ps.tile([128, 512], f32, name="wops", tag="pops")
            for es in range(ES):
                nc.tensor.matmul(
                    pk[:],
                    lhsT=wqk_sb[es][:, ct * 128 : (ct + 1) * 128],
                    rhs=xe_sb[es][:, ssl],
                    start=(es == 0),
                    stop=(es == ES - 1),
                )
            dst = k_sb[1] if ct == 2 else q_sb[1]
            nc.vector.tensor_copy(dst[:, ssl], pk[:])

        # ---- phase C: v projection ---------------------------------------
        # v[s, c] = sum_e xdT[e, s] * wvT[e, c]; store interleaved (h, 128)
        # with a ones column at [h, 64] for the softmax denominator.
        # xd streamed in quad-tiles [128, 512] covering 4 s-tiles each.
        for stq in range(4):
            xd_q = []
            for es in range(ES):
                xt = xd_pool.tile([128, 512], bf16, name="xdt")
                nc.sync.dma_start(
                    xt[:],
                    xdT_d[es * 128 : (es + 1) * 128, stq * 512 : stq * 512 + 512],
                )
                xd_q.append(xt)
            # trickle in the first mask i-halves so attention isn't gated on
            # mask arrival when phase C finishes
            for jt in range(stq * 4, stq * 4 + 4):
                nc.sync.dma_start(
                    mask_sb[jt][:, 0:1024], maskT_d[jt * 128 : (jt + 1) * 128, 0:1024]
                )
            for k in range(4):
                st = stq * 4 + k
                pv = pv_ps.tile([128, 256], f32, name="vprojps", tag="pvps")
                for es in range(ES):
                    nc.tensor.matmul(
                        pv[:],
                        lhsT=xd_q[es][:, k * 128 : (k + 1) * 128],
                        rhs=wv_sb[es][:],
                        start=(es == 0),
                        stop=(es == ES - 1),
                    )
                view = v_sb[st].rearrange("p (h c) -> p h c", c=128)
                if st % 2 == 0:
                    nc.vector.tensor_copy(
                        view[:, :, 0:64], pv[:].rearrange("p (h c) -> p h c", c=64)
                    )
                else:
                    nc.scalar.activation(
                        view[:, :, 0:64],
                        pv[:].rearrange("p (h c) -> p h c", c=64),
                        Copy,
                    )
                nc.gpsimd.memset(view[:, :, 64:65], 1.0)
                nc.gpsimd.memset(view[:, :, 65:128], 0.0)

        # second mask i-halves (needed from the ib=2 attention pass onward)
        for jt in range(JT):
            nc.sync.dma_start(
                mask_sb[jt][:, 1024:2048], maskT_d[jt * 128 : (jt + 1) * 128, 1024:2048]
            )

        # ---- phase D: attention (+ phase E interleaved in hp=1 pass) -----
        def wo_unit(et, ib2, split_dma=False):
            po = po_ps.tile([128, 512], f32, name="wops", tag="pops")
            for hp2 in range(2):
                nc.tensor.matmul(
                    po[:],
                    lhsT=wo_sb[hp2][:, et * 128 : (et + 1) * 128],
                    rhs=vals_sb[hp2][:, ib2 * 512 : ib2 * 512 + 512],
                    start=(hp2 == 0),
                    stop=(hp2 == 1),
                )
            ot = out_pool.tile([128, 512], bf16, name="ot")
            nc.vector.tensor_copy(ot[:], po[:])
            nc.sync.dma_start(
                outT_d[et * 128 : (et + 1) * 128, ib2 * 512 : ib2 * 512 + 512],
                ot[:],
            )

        def qk_mms(hp, ib, jt):
            isl = slice(ib * 512, ib * 512 + 512)
            jsl = slice(jt * 128, jt * 128 + 128)
            qk_t = qk_ps.tile([128, 1024], f32, name="qkt", tag="qkps")
            nc.tensor.matmul(
                qk_t[:, 0:512],
                lhsT=k_sb[hp][0:64, jsl],
                rhs=q_sb[hp][0:64, isl],
                start=True,
                stop=True,
                tile_position=(0, 0),
            )
            nc.tensor.matmul(
                qk_t[:, 512:1024],
                lhsT=k_sb[hp][64:128, jsl],
                rhs=q_sb[hp][64:128, isl],
                start=True,
                stop=True,
                tile_position=(64, 0),
            )
            return qk_t

        for hp in range(2):
            for ib in range(IB):
                isl = slice(ib * 512, ib * 512 + 512)
                pvA = pv_ps.tile([128, 512], f32, name="pvA", tag="pvps")
                pvB = pv_ps.tile([128, 512], f32, name="pvB", tag="pvps")
                for jt in range(JT):
                    qk_t = qk_mms(hp, ib, jt)
                    ex_t = ex_pool.tile([128, 1024], bf16, name="ext")
                    nc.scalar.activation(
                        ex_t[:],
                        qk_t[:],
                        mybir.ActivationFunctionType.Exp,
                        bias=exp_bias[:],
                        scale=NEG_SCALE,
                    )
                    exv = ex_t.rearrange("p (h c) -> p h c", c=512)
                    nc.vector.tensor_mul(
                        exv[:, :, :],
                        exv[:, :, :],
                        mask_sb[jt][:, isl].rearrange("p (h c) -> p h c", c=512)
                        .broadcast_to([128, 2, 512]),
                    )
                    vview = v_sb[jt].rearrange("p (h c) -> p h c", c=128)
                    nc.tensor.matmul(
                        pvA[:],
                        lhsT=vview[:, 2 * hp, :],
                        rhs=ex_t[:, 0:512],
                        start=(jt == 0),
                        stop=(jt == JT - 1),
                    )
                    nc.tensor.matmul(
                        pvB[:],
                        lhsT=vview[:, 2 * hp + 1, :],
                        rhs=ex_t[:, 512:1024],
                        start=(jt == 0),
                        stop=(jt == JT - 1),
                    )
                    # interleave filler into the PE slack under the
                    # scalar-bound softmax: hp=1 projection units during
                    # hp=0, previous i-block's W_o units during hp=1
                    if hp == 0 and jt in (5, 10):
                        b1_unit(ib * 2 + (0 if jt == 5 else 1))
                    if hp == 1 and ib > 0 and jt in WO_JTS:
                        wo_unit(WO_JTS.index(jt), ib - 1)
                # normalize: vals = pv[0:65] / pv[64]. First drain pv[0:65]
                # to SBUF (scalar head A, vector head B) so the psum banks
                # free up immediately and the next i-block's PV accumulation
                # isn't stalled on the normalization chain.
                pvsA = div_pool.tile([128, 512], f32, name="pvsA")
                pvsB = div_pool.tile([128, 512], f32, name="pvsB")
                nc.scalar.activation(pvsA[0:65, :], pvA[0:65, :], Copy)
                nc.vector.tensor_copy(pvsB[0:65, :], pvB[0:65, :])
                # rowsums live at SBUF partition 64; bounce through DRAM to
                # broadcast across partitions (SBUF sources can't have a
                # zero partition stride), take reciprocal, then multiply
                # (the head-B result is moved to partitions 64..127 by DMA,
                # which shifts freely).
                scr = rs_scr[hp * IB + ib]
                nc.sync.dma_start(scr[0:1, 0:512], pvsA[64:65, :])
                nc.sync.dma_start(scr[0:1, 512:1024], pvsB[64:65, :])
                for h2, pvs in ((0, pvsA), (1, pvsB)):
                    rb = div_pool.tile([64, 512], f32, name="rb")
                    nc.sync.dma_start(
                        rb[:],
                        scr[0:1, h2 * 512 : h2 * 512 + 512].broadcast_to([64, 512]),
                    )
                    rc = div_pool.tile([64, 512], f32, name="rc")
                    nc.vector.reciprocal_approx_fast(rc[:], rb[:])
                    if h2 == 0:
                        nc.vector.tensor_mul(
                            vals_sb[hp][0:64, isl], pvs[0:64, :], rc[:]
                        )
                    else:
                        vtmp = div_pool.tile([64, 512], bf16, name="vtmp")
                        nc.vector.tensor_mul(vtmp[:], pvs[0:64, :], rc[:])
                        nc.sync.dma_start(
                            vals_sb[hp][64:128, isl], vtmp[:]
                        )

        # ---- phase E remainder: W_o for the last i-block -----------------
        # double-buffered via the (now free) qk psum pool, drains split
        # scalar/vector so the tail isn't serialized on one engine
        for et in range(ET):
            po2 = qk_ps.tile([128, 1024], f32, name="wops2", tag="qkps")
            for hp2 in range(2):
                nc.tensor.matmul(
                    po2[:, 0:512],
                    lhsT=wo_sb[hp2][:, et * 128 : (et + 1) * 128],
                    rhs=vals_sb[hp2][:, (IB - 1) * 512 : IB * 512],
                    start=(hp2 == 0),
                    stop=(hp2 == 1),
                )
            ot = out_pool.tile([128, 512], bf16, name="ot")
            if et % 2 == 0:
                nc.vector.tensor_copy(ot[:], po2[:, 0:512])
            else:
                nc.scalar.activation(ot[:], po2[:, 0:512], Copy)
            nc.sync.dma_start(
                outT_d[et * 128 : (et + 1) * 128, (IB - 1) * 512 : IB * 512],
                ot[:],
            )

    nc.compile()
    return nc


_NC_CACHE = None


def _get_nc():
    global _NC_CACHE
    if _NC_CACHE is None:
        _NC_CACHE = build_nc()
    return _NC_CACHE


def _bf16(a):
    try:
        import ml_dtypes

        return np.asarray(a, dtype=ml_dtypes.bfloat16)
    except ImportError:
        import jax.numpy as jnp

        return np.asarray(jnp.asarray(a, dtype=jnp.bfloat16))


def shard_inputs(x_encoder, x_decoder, mask, W_qk, W_v, W_o):
    """Build the 8 per-core input maps."""
    x_encoder = np.asarray(x_encoder)
    x_decoder = np.asarray(x_decoder)
    mask = np.asarray(mask)
    W_qk = np.asarray(W_qk)
    W_v = np.asarray(W_v)
    W_o = np.asarray(W_o)

    xeT = [_bf16(np.ascontiguousarray(x_encoder[b].T)) for b in range(N_BATCH)]
    xdT = [_bf16(np.ascontiguousarray(x_decoder[b].T)) for b in range(N_BATCH)]
    maskT = _bf16(np.ascontiguousarray(mask.T))

    wqkT_g = []
    wvT_g = []
    woT_g = []
    for g in range(G):
        rows = []
        for hp in range(2):
            h0 = 4 * g + 2 * hp
            h1 = h0 + 1
            # k chans for the pair, then q chans (matches ct order)
            rows.append(W_qk[128 * h0 + 64 : 128 * h0 + 128])
            rows.append(W_qk[128 * h1 + 64 : 128 * h1 + 128])
            rows.append(W_qk[128 * h0 : 128 * h0 + 64])
            rows.append(W_qk[128 * h1 : 128 * h1 + 64])
        # reorder so layout is [k0,k1 | q0,q1 | k2,k3 | q2,q3] along cols
        sel = np.concatenate(
            [rows[0], rows[1], rows[2], rows[3], rows[4], rows[5], rows[6], rows[7]],
            axis=0,
        )
        wqkT_g.append(_bf16(np.ascontiguousarray(sel.T)))
        wvT_g.append(_bf16(np.ascontiguousarray(W_v[256 * g : 256 * g + 256, :].T)))
        woT_g.append(_bf16(np.ascontiguousarray(W_o[:, 256 * g : 256 * g + 256].T)))

    in_maps = []
    for c in range(NC):
        b, g = c // G, c % G
        in_maps.append(
            {
                "xeT": xeT[b],
                "xdT": xdT[b],
                "wqkT": wqkT_g[g],
                "wvT": wvT_g[g],
                "woT": woT_g[g],
                "maskT": maskT,
            }
        )
    return in_maps


def gather_outputs(results):
    """Sum per-group partials and transpose back to (N, S, E)."""
    out = np.empty((N_BATCH, S, E), dtype=np.float32)
    for b in range(N_BATCH):
        acc = results[b * G]["outT"].astype(np.float32)
        for g in range(1, G):
            acc += results[b * G + g]["outT"].astype(np.float32)
        out[b] = acc.T
    return out


def kernel(x_encoder, x_decoder, mask, W_qk, W_v, W_o):
    nc = _get_nc()
    in_maps = shard_inputs(x_encoder, x_decoder, mask, W_qk, W_v, W_o)
    res = bass_utils.run_bass_kernel_spmd(
        nc, in_maps, core_ids=list(range(NC)), trace=False
    )
    kernel.last_results = res
    return gather_outputs(res.results)
